# revision 3
# baseline (speedup 1.0000x reference)
"""GMM negative log-likelihood on 8 TRN2 NeuronCores.

score[n, m] = wlog[m] - qf[n, m] factors exactly as F[n, :6] @ C[:6, m]
with features F = [1, x, y, x^2, xy, y^2], so the O(N*M) work is a
K=6 matmul on the TensorEngine followed by exp + row-sum (fused on the
scalar engine via accum_out) and a log at the end.  Data-parallel over
the N=65536 samples: each core gets 8192 samples and the full C.

Matmuls run in bf16 (1 cycle/row vs 4 for f32); each 128-sample tile's
[128, 1024] scores live in one 2-bank PSUM tile so a single in-place
Exp activation with accum_out covers the whole tile.
"""

import numpy as np

import concourse.bacc as bacc
import concourse.bass as bass
import concourse.mybir as mybir
import concourse.tile as tile
from concourse.bass_utils import run_bass_kernel_spmd

N, M, NCORES = 65536, 1024, 8
NSH = N // NCORES          # 8192 samples per core
P = 128                    # partitions per tile
NT = NSH // P              # 64 tiles per core
HALF = M // 2              # 512 = max moving free dim per matmul

_cache = {}


def _build():
    f32 = mybir.dt.float32
    bf16 = mybir.dt.bfloat16
    nc = bacc.Bacc(None, target_bir_lowering=False)

    ft_d = nc.declare_dram_parameter("ft", [6, NSH], bf16, isOutput=False)
    c_d = nc.declare_dram_parameter("cmat", [6, M], bf16, isOutput=False)
    out_d = nc.declare_dram_parameter("out", [P, 1], f32, isOutput=True)

    with tile.TileContext(nc) as tc:
        with (
            tc.tile_pool(name="const", bufs=1) as const,
            tc.tile_pool(name="psum", bufs=3, space=bass.MemorySpace.PSUM) as psum,
        ):
            ft = const.tile([6, NSH], bf16)
            cmat = const.tile([6, M], bf16)
            nc.sync.dma_start(out=ft[:], in_=ft_d[:])
            nc.sync.dma_start(out=cmat[:], in_=c_d[:])

            s_all = const.tile([P, NT], f32)  # per-tile sum(exp(scores))

            for t in range(NT):
                lhsT = ft[:, t * P:(t + 1) * P]
                pt = psum.tile([P, M], f32, tag="pt")
                nc.tensor.matmul(pt[:, 0:HALF], lhsT, cmat[:, 0:HALF])
                nc.tensor.matmul(pt[:, HALF:M], lhsT, cmat[:, HALF:M])
                nc.scalar.activation(
                    pt[:], pt[:], mybir.ActivationFunctionType.Exp,
                    accum_out=s_all[:, t:t + 1],
                )

            ll = const.tile([P, NT], f32)
            nc.scalar.activation(ll[:], s_all[:], mybir.ActivationFunctionType.Ln)
            red = const.tile([P, 1], f32)
            nc.vector.reduce_sum(red[:], ll[:], axis=mybir.AxisListType.X)
            nc.sync.dma_start(out=out_d[:], in_=red[:])

    nc.compile()
    return nc


def kernel(sample, mu, sigma_log, theta, w):
    x = sample[:, 0].astype(np.float64)
    y = sample[:, 1].astype(np.float64)
    mux = mu[:, 0].astype(np.float64)
    muy = mu[:, 1].astype(np.float64)
    sl = sigma_log.astype(np.float64)
    th = theta.astype(np.float64)
    wv = w[:, 0].astype(np.float64)

    a = np.exp(-2.0 * sl[:, 0])
    b = np.exp(-2.0 * sl[:, 1])
    c, s = np.cos(th), np.sin(th)
    g11 = a * c * c + b * s * s
    g12 = (a - b) * c * s
    g22 = a * s * s + b * c * c
    wmax = wv.max()
    wlog = (wv - (wmax + np.log(np.exp(wv - wmax).sum()))) - sl.sum(axis=1)

    # score = F @ C with F = [1, x, y, x^2, xy, y^2]
    cm = np.stack([
        wlog - (g11 * mux * mux + 2.0 * g12 * mux * muy + g22 * muy * muy),
        2.0 * (g11 * mux + g12 * muy),
        2.0 * (g12 * mux + g22 * muy),
        -g11,
        -2.0 * g12,
        -g22,
    ]).astype(np.float32)
    ftf = np.stack([np.ones_like(x), x, y, x * x, x * y, y * y]).astype(np.float32)

    import ml_dtypes
    cm16 = cm.astype(ml_dtypes.bfloat16)
    ftf16 = ftf.astype(ml_dtypes.bfloat16)

    if "nc" not in _cache:
        _cache["nc"] = _build()
    nc = _cache["nc"]

    in_maps = [
        {"ft": np.ascontiguousarray(ftf16[:, i * NSH:(i + 1) * NSH]), "cmat": cm16}
        for i in range(NCORES)
    ]
    res = run_bass_kernel_spmd(nc, in_maps, core_ids=list(range(NCORES)))
    _cache["last_result"] = res
    total = np.float64(0.0)
    for r in res.results:
        total += np.asarray(r["out"], dtype=np.float64).sum()
    return np.float32(-total)


# revision 18
# speedup vs baseline: 1.2081x; 1.2081x over previous
"""GMM negative log-likelihood on 8 TRN2 NeuronCores.

score[n, m] = wlog[m] - qf[n, m] factors exactly as F[n, :6] @ C[:6, m]
with features F = [1, x, y, x^2, xy, y^2], so the O(N*M) work is a
K=6 bf16 matmul on the TensorEngine.  Per 128-sample tile the [128,1024]
scores land in a 2-bank PSUM tile; columns [0,CA) get a true Exp with
fused row-sum on the scalar engine (in-place, accum_out), columns
[CA,1024) get a Schraudolph fast-exp (affine in f32 -> int32 cast =
exponent/mantissa bit construction) on the GPSIMD engine, row-summed on
the vector engine, splitting the exp work across three engines.

Inputs arrive as one [102, 3072] bf16 blob: the feature rows live at
partition groups {0,32,64,96} (PE quadrant-aligned) with C replicated
per group, so the DMA uses 24 partitions instead of 6 and the first
matmul can start after a ~2KB/partition leading transfer.
Data-parallel over N: each core gets 8192 samples and the full C.
"""

import numpy as np

import concourse.bacc as bacc
import concourse.bass as bass
import concourse.mybir as mybir
import concourse.tile as tile
from concourse.bass_utils import run_bass_kernel_spmd

N, M, NCORES = 65536, 1024, 8
NSH = N // NCORES          # 8192 samples per core
P = 128                    # partitions per tile
NT = NSH // P              # 64 tiles per core
NG = 4                     # partition groups for features
GSH = NSH // NG            # 2048 samples per group
NC_LOC = GSH // P          # 16 local col blocks per group
HALF = M // 2              # 512 = max moving free dim per matmul
BLOBW = M + GSH            # 3072 blob columns: [cmat | features]
CA = 704                   # columns handled by scalar-engine true exp

# Schraudolph: exp(s) ~= bitcast_f32(int32(A*s + B)), A = 2^23/ln2.
# B = 2^23*(127 - c) with c = log2(mean_f (1+f)*2^-f) making the
# relative error zero-mean over uniform mantissa fractions.
_SCH_A = float(2 ** 23 / np.log(2.0))
_SCH_C = float(np.log2(np.mean((1.0 + np.linspace(0, 1, 4097)) * 2.0 ** -np.linspace(0, 1, 4097))))
_SCH_B = float(2 ** 23 * (127.0 - _SCH_C))

_cache = {}


def _build(ca=CA):
    f32 = mybir.dt.float32
    i32 = mybir.dt.int32
    bf16 = mybir.dt.bfloat16
    nc = bacc.Bacc(None, target_bir_lowering=False)

    blob_d = nc.declare_dram_parameter("blob", [102, BLOBW], bf16, isOutput=False)
    out_d = nc.declare_dram_parameter("out", [P, 1], f32, isOutput=True)

    with tile.TileContext(nc) as tc:
        with (
            tc.tile_pool(name="const", bufs=1) as const,
            tc.tile_pool(name="psum", bufs=4, space=bass.MemorySpace.PSUM) as psum,
        ):
            blob = const.tile([102, BLOBW], bf16)
            # staged: [cmat | first col block] first so compute starts early
            nc.sync.dma_start(out=blob[:, 0:M + P], in_=blob_d[:, 0:M + P])
            nc.sync.dma_start(out=blob[:, M + P:M + 5 * P], in_=blob_d[:, M + P:M + 5 * P])
            nc.sync.dma_start(out=blob[:, M + 5 * P:BLOBW], in_=blob_d[:, M + 5 * P:BLOBW])

            sa = const.tile([P, NT], f32)  # ACT partial row-sums
            sd = const.tile([P, NT], f32)  # DVE partial row-sums

            # ACT: true exp + fused row-sum on [0, ca).
            # DVE: Schraudolph fast-exp on [ca, M) in place + row-sum
            # (deferred one tile so its input's write-ack has returned).
            pend = None  # (slice AP, tile idx) awaiting DVE reduce
            t = 0
            for c in range(NC_LOC):
                for g in range(NG):
                    gp = 32 * g
                    cmat = blob[gp:gp + 6, 0:M]
                    lhsT = blob[gp:gp + 6, M + c * P:M + (c + 1) * P]
                    tp = (gp, 0)
                    pt = psum.tile([P, M], f32, tag="pt")
                    nc.tensor.matmul(pt[:, 0:HALF], lhsT, cmat[:, 0:HALF],
                                     tile_position=tp)
                    nc.tensor.matmul(pt[:, HALF:M], lhsT, cmat[:, HALF:M],
                                     tile_position=tp)
                    nc.scalar.activation(
                        pt[:, 0:ca], pt[:, 0:ca], mybir.ActivationFunctionType.Exp,
                        accum_out=sa[:, t:t + 1],
                    )
                    sl_f = pt[:, ca:M]
                    nc.vector.tensor_scalar(
                        out=sl_f.bitcast(i32), in0=sl_f, scalar1=_SCH_A, scalar2=_SCH_B,
                        op0=mybir.AluOpType.mult, op1=mybir.AluOpType.add,
                    )
                    if pend is not None:
                        pf, pi = pend
                        nc.vector.reduce_sum(sd[:, pi:pi + 1], pf, axis=mybir.AxisListType.X)
                    pend = (sl_f, t)
                    t += 1
            pf, pi = pend
            nc.vector.reduce_sum(sd[:, pi:pi + 1], pf, axis=mybir.AxisListType.X)

            stot = const.tile([P, NT], f32)
            nc.vector.tensor_tensor(
                out=stot[:], in0=sa[:], in1=sd[:], op=mybir.AluOpType.add
            )
            ll = const.tile([P, NT], f32)
            nc.scalar.activation(ll[:], stot[:], mybir.ActivationFunctionType.Ln)
            red = const.tile([P, 1], f32)
            nc.vector.reduce_sum(red[:], ll[:], axis=mybir.AxisListType.X)
            nc.sync.dma_start(out=out_d[:], in_=red[:])

    nc.compile()
    return nc


def kernel(sample, mu, sigma_log, theta, w):
    import ml_dtypes

    x = sample[:, 0].astype(np.float64)
    y = sample[:, 1].astype(np.float64)
    mux = mu[:, 0].astype(np.float64)
    muy = mu[:, 1].astype(np.float64)
    sl = sigma_log.astype(np.float64)
    th = theta.astype(np.float64)
    wv = w[:, 0].astype(np.float64)

    a = np.exp(-2.0 * sl[:, 0])
    b = np.exp(-2.0 * sl[:, 1])
    c, s = np.cos(th), np.sin(th)
    g11 = a * c * c + b * s * s
    g12 = (a - b) * c * s
    g22 = a * s * s + b * c * c
    wmax = wv.max()
    wlog = (wv - (wmax + np.log(np.exp(wv - wmax).sum()))) - sl.sum(axis=1)

    # score = F @ C with F = [1, x, y, x^2, xy, y^2]
    cm = np.stack([
        wlog - (g11 * mux * mux + 2.0 * g12 * mux * muy + g22 * muy * muy),
        2.0 * (g11 * mux + g12 * muy),
        2.0 * (g12 * mux + g22 * muy),
        -g11,
        -2.0 * g12,
        -g22,
    ]).astype(np.float32)
    ftf = np.stack([np.ones_like(x), x, y, x * x, x * y, y * y]).astype(np.float32)

    cm16 = cm.astype(ml_dtypes.bfloat16)
    ftf16 = ftf.astype(ml_dtypes.bfloat16)

    if "nc" not in _cache:
        _cache["nc"] = _build()
    nc = _cache["nc"]

    in_maps = []
    for i in range(NCORES):
        blob = np.zeros((102, BLOBW), dtype=ml_dtypes.bfloat16)
        base = i * NSH
        for g in range(NG):
            gp = 32 * g
            blob[gp:gp + 6, 0:M] = cm16
            blob[gp:gp + 6, M:BLOBW] = ftf16[:, base + g * GSH:base + (g + 1) * GSH]
        in_maps.append({"blob": blob})
    res = run_bass_kernel_spmd(nc, in_maps, core_ids=list(range(NCORES)))
    _cache["last_result"] = res
    total = np.float64(0.0)
    for r in res.results:
        total += np.asarray(r["out"], dtype=np.float64).sum()
    return np.float32(-total)


# revision 19
# speedup vs baseline: 1.4754x; 1.2213x over previous
"""V5: GMM NLL with PE-reduced Schraudolph slice.

Per 512-sample quad:
- sample-major: 4 matmuls [128sm, 512] (comps 0..511) -> ACT true exp
  + fused row-sum (accum_out) -> sa.
- flipped: 4 chunk matmuls [128comp, 512sm] (comps 512..1023) -> DVE
  Schraudolph affine+cast to int16 = bf16 exp bits -> SBUF ebits ->
  16 tiny ones-matmuls (lhsT = ebits block, rhs = ones[128,1]) PSUM-
  accumulate per-sample sums into a persistent acc bank.
ACT ~757ns, DVE ~596ns, PE ~500ns per 128-sample tile.
"""

import numpy as np

import concourse.bacc as bacc
import concourse.bass as bass
import concourse.mybir as mybir
import concourse.tile as tile
from concourse.bass_utils import run_bass_kernel_spmd

N, M, NCORES = 65536, 1024, 8
NSH = N // NCORES          # 8192 samples per core
P = 128
NT = NSH // P              # 64 tiles per core
NG = 4                     # feature partition groups
GSH = NSH // NG            # 2048 samples per group
HALF = M // 2              # 512
BLOBW = M + GSH            # 3072
CA = 512                   # true-exp comps (sample-major slice)
QS = 512                   # samples per quad
NCH = (M - CA) // P        # 4 Schraudolph comp chunks

# bf16-bit Schraudolph: bits16(exp(s)) ~= int16(A16*s + B16)
_SCH_C = float(np.log2(np.mean((1.0 + np.linspace(0, 1, 4097)) * 2.0 ** -np.linspace(0, 1, 4097))))
_SCH_A16 = float(2 ** 7 / np.log(2.0))
_SCH_B16 = float(2 ** 7 * (127.0 - _SCH_C))

_cache = {}


def _build(ca=CA):
    f32 = mybir.dt.float32
    i16 = mybir.dt.int16
    bf16 = mybir.dt.bfloat16
    nc = bacc.Bacc(None, target_bir_lowering=False)

    blob_d = nc.declare_dram_parameter("blob", [102, BLOBW], bf16, isOutput=False)
    out_d = nc.declare_dram_parameter("out", [P, 1], f32, isOutput=True)

    with tile.TileContext(nc) as tc:
        with (
            tc.tile_pool(name="const", bufs=1) as const,
            tc.tile_pool(name="ebuf", bufs=2) as ebuf_pool,
            tc.tile_pool(name="psa", bufs=3, space=bass.MemorySpace.PSUM) as psa,
            tc.tile_pool(name="psb", bufs=3, space=bass.MemorySpace.PSUM) as psb,
            tc.tile_pool(name="pacc", bufs=1, space=bass.MemorySpace.PSUM) as pacc,
            tc.tile_pool(name="pwarm", bufs=1, space=bass.MemorySpace.PSUM) as pwarm,
        ):
            # blob columns: [cmatA(0:512) | F quad0(512:1024) | cmatB(1024:1536)
            #                | F quads 1-3 (1536:3072)], per partition group.
            blob = const.tile([102, BLOBW], bf16)
            nc.sync.dma_start(out=blob[:, 0:ca + P], in_=blob_d[:, 0:ca + P])
            nc.sync.dma_start(out=blob[:, ca + P:3 * HALF], in_=blob_d[:, ca + P:3 * HALF])
            nc.sync.dma_start(out=blob[:, 3 * HALF:BLOBW], in_=blob_d[:, 3 * HALF:BLOBW])

            # PE p-state warm-up: small data-independent matmuls keep the
            # tensor engine busy from t~0 so real matmuls run at full clock
            # once the first DMA lands (~2.8us); each is short so the real
            # work is delayed by at most one of them.
            warm = const.tile([6, P], bf16)
            nc.vector.memset(warm[:], 0.0)
            ones = const.tile([P, 1], bf16)
            nc.vector.memset(ones[:], 1.0)
            wp = pwarm.tile([P, P], f32, tag="warm")
            for _ in range(16):
                nc.tensor.matmul(wp[:], warm[:], warm[:], tile_position=(0, 0))

            sa = const.tile([P, NT], f32)    # ACT partial row-sums
            acc = pacc.tile([P, NT], f32)    # PE-accumulated Schraudolph sums

            def fcol(c4):  # start column of quad c4's features
                return ca if c4 == 0 else 3 * HALF + (c4 - 1) * QS

            t = 0
            pend_ones = None
            for c4 in range(GSH // QS):       # 4 quads per group
                for g in range(NG):
                    gp = 32 * g
                    tp = (gp, 0)
                    feat = blob[gp:gp + 6, fcol(c4):fcol(c4) + QS]
                    # sample-major matmuls + ACT exp/accum per tile (first:
                    # they feed the bottleneck engine)
                    for j in range(QS // P):
                        lhsT = blob[gp:gp + 6, fcol(c4) + j * P:fcol(c4) + (j + 1) * P]
                        pa = psa.tile([P, ca], f32, tag="pa")
                        nc.tensor.matmul(pa[:], lhsT, blob[gp:gp + 6, 0:ca],
                                         tile_position=tp)
                        nc.scalar.activation(
                            pa[:], pa[:], mybir.ActivationFunctionType.Exp,
                            accum_out=sa[:, t + j:t + j + 1],
                        )
                    # flipped chunk matmuls -> single-chunk psum tiles
                    pbs = []
                    for k in range(NCH):
                        pb = psb.tile([P, QS], f32, tag="pb")
                        cchunk = blob[gp:gp + 6, 2 * HALF + k * P:2 * HALF + (k + 1) * P]
                        nc.tensor.matmul(pb[:], cchunk, feat, tile_position=tp)
                        pbs.append(pb)
                    # DVE Schraudolph: f32 -> bf16 bits (int16)
                    eb = ebuf_pool.tile([P, NCH, QS], i16, tag="eb")
                    for k in range(NCH):
                        nc.vector.tensor_scalar(
                            out=eb[:, k, :], in0=pbs[k][:],
                            scalar1=_SCH_A16, scalar2=_SCH_B16,
                            op0=mybir.AluOpType.mult, op1=mybir.AluOpType.add,
                        )
                    # PE ones-matmuls for the PREVIOUS quad (sems long ready)
                    if pend_ones is not None:
                        peb, pt0 = pend_ones
                        ebv = peb.bitcast(bf16)
                        for j in range(QS // P):
                            for k in range(NCH):
                                nc.tensor.matmul(
                                    acc[:, pt0 + j:pt0 + j + 1],
                                    ebv[:, k, j * P:(j + 1) * P], ones[:],
                                    start=(k == 0), stop=(k == NCH - 1),
                                    tile_position=(0, 0),
                                    skip_group_check=True,
                                )
                    pend_ones = (eb, t)
                    t += QS // P
            peb, pt0 = pend_ones
            ebv = peb.bitcast(bf16)
            for j in range(QS // P):
                for k in range(NCH):
                    nc.tensor.matmul(
                        acc[:, pt0 + j:pt0 + j + 1],
                        ebv[:, k, j * P:(j + 1) * P], ones[:],
                        start=(k == 0), stop=(k == NCH - 1),
                        tile_position=(0, 0),
                        skip_group_check=True,
                    )

            stot = const.tile([P, NT], f32)
            nc.vector.tensor_tensor(
                out=stot[:], in0=sa[:], in1=acc[:], op=mybir.AluOpType.add
            )
            ll = const.tile([P, NT], f32)
            nc.scalar.activation(ll[:], stot[:], mybir.ActivationFunctionType.Ln)
            red = const.tile([P, 1], f32)
            nc.vector.reduce_sum(red[:], ll[:], axis=mybir.AxisListType.X)
            nc.sync.dma_start(out=out_d[:], in_=red[:])

    nc.compile()
    return nc


def kernel(sample, mu, sigma_log, theta, w):
    import ml_dtypes

    x = sample[:, 0].astype(np.float64)
    y = sample[:, 1].astype(np.float64)
    mux = mu[:, 0].astype(np.float64)
    muy = mu[:, 1].astype(np.float64)
    sl = sigma_log.astype(np.float64)
    th = theta.astype(np.float64)
    wv = w[:, 0].astype(np.float64)

    a = np.exp(-2.0 * sl[:, 0])
    b = np.exp(-2.0 * sl[:, 1])
    c, s = np.cos(th), np.sin(th)
    g11 = a * c * c + b * s * s
    g12 = (a - b) * c * s
    g22 = a * s * s + b * c * c
    wmax = wv.max()
    wlog = (wv - (wmax + np.log(np.exp(wv - wmax).sum()))) - sl.sum(axis=1)

    cm = np.stack([
        wlog - (g11 * mux * mux + 2.0 * g12 * mux * muy + g22 * muy * muy),
        2.0 * (g11 * mux + g12 * muy),
        2.0 * (g12 * mux + g22 * muy),
        -g11,
        -2.0 * g12,
        -g22,
    ]).astype(np.float32)
    ftf = np.stack([np.ones_like(x), x, y, x * x, x * y, y * y]).astype(np.float32)

    cm16 = cm.astype(ml_dtypes.bfloat16)
    ftf16 = ftf.astype(ml_dtypes.bfloat16)

    if "nc" not in _cache:
        _cache["nc"] = _build()
    nc = _cache["nc"]

    in_maps = []
    for i in range(NCORES):
        blob = np.zeros((102, BLOBW), dtype=ml_dtypes.bfloat16)
        base = i * NSH
        for g in range(NG):
            gp = 32 * g
            fg = ftf16[:, base + g * GSH:base + (g + 1) * GSH]
            blob[gp:gp + 6, 0:CA] = cm16[:, 0:CA]              # cmatA
            blob[gp:gp + 6, CA:2 * HALF] = fg[:, 0:QS]         # F quad 0
            blob[gp:gp + 6, 2 * HALF:3 * HALF] = cm16[:, CA:M]  # cmatB
            blob[gp:gp + 6, 3 * HALF:BLOBW] = fg[:, QS:GSH]    # F quads 1-3
        in_maps.append({"blob": blob})
    res = run_bass_kernel_spmd(nc, in_maps, core_ids=list(range(NCORES)))
    _cache["last_result"] = res
    total = np.float64(0.0)
    for r in res.results:
        total += np.asarray(r["out"], dtype=np.float64).sum()
    return np.float32(-total)


# revision 22
# speedup vs baseline: 1.5136x; 1.0258x over previous
"""V5: GMM NLL with PE-reduced Schraudolph slice.

Per 512-sample quad:
- sample-major: 4 matmuls [128sm, 512] (comps 0..511) -> ACT true exp
  + fused row-sum (accum_out) -> sa.
- flipped: 4 chunk matmuls [128comp, 512sm] (comps 512..1023) -> DVE
  Schraudolph affine+cast to int16 = bf16 exp bits -> SBUF ebits ->
  16 tiny ones-matmuls (lhsT = ebits block, rhs = ones[128,1]) PSUM-
  accumulate per-sample sums into a persistent acc bank.
ACT ~757ns, DVE ~596ns, PE ~500ns per 128-sample tile.
"""

import numpy as np

import concourse.bacc as bacc
import concourse.bass as bass
import concourse.mybir as mybir
import concourse.tile as tile
from concourse.bass_utils import run_bass_kernel_spmd

N, M, NCORES = 65536, 1024, 8
NSH = N // NCORES          # 8192 samples per core
P = 128
NT = NSH // P              # 64 tiles per core
NG = 4                     # feature partition groups
GSH = NSH // NG            # 2048 samples per group
HALF = M // 2              # 512
BLOBW = M + GSH            # 3072
CA = 512                   # true-exp comps (sample-major slice)
QS = 512                   # samples per quad
NCH = (M - CA) // P        # 4 Schraudolph comp chunks

# bf16-bit Schraudolph: bits16(exp(s)) ~= int16(A16*s + B16)
_SCH_C = float(np.log2(np.mean((1.0 + np.linspace(0, 1, 4097)) * 2.0 ** -np.linspace(0, 1, 4097))))
_SCH_A16 = float(2 ** 7 / np.log(2.0))
_SCH_B16 = float(2 ** 7 * (127.0 - _SCH_C))

_cache = {}


def _build(ca=CA):
    f32 = mybir.dt.float32
    i16 = mybir.dt.int16
    bf16 = mybir.dt.bfloat16
    nc = bacc.Bacc(None, target_bir_lowering=False)

    blob_d = nc.declare_dram_parameter("blob", [102, BLOBW], bf16, isOutput=False)
    out_d = nc.declare_dram_parameter("out", [P, NT], f32, isOutput=True)

    with tile.TileContext(nc) as tc:
        with (
            tc.tile_pool(name="const", bufs=1) as const,
            tc.tile_pool(name="ebuf", bufs=2) as ebuf_pool,
            tc.tile_pool(name="psa", bufs=3, space=bass.MemorySpace.PSUM) as psa,
            tc.tile_pool(name="psb", bufs=3, space=bass.MemorySpace.PSUM) as psb,
            tc.tile_pool(name="pacc", bufs=1, space=bass.MemorySpace.PSUM) as pacc,
            tc.tile_pool(name="pwarm", bufs=1, space=bass.MemorySpace.PSUM) as pwarm,
        ):
            # blob columns: [cmatA(0:512) | F quad0(512:1024) | cmatB(1024:1536)
            #                | F quads 1-3 (1536:3072)], per partition group.
            blob = const.tile([102, BLOBW], bf16)
            nc.sync.dma_start(out=blob[:, 0:ca + P], in_=blob_d[:, 0:ca + P])
            nc.sync.dma_start(out=blob[:, ca + P:3 * HALF], in_=blob_d[:, ca + P:3 * HALF])
            nc.sync.dma_start(out=blob[:, 3 * HALF:BLOBW], in_=blob_d[:, 3 * HALF:BLOBW])

            # PE p-state warm-up: small data-independent matmuls keep the
            # tensor engine busy from t~0 so real matmuls run at full clock
            # once the first DMA lands (~2.8us); each is short so the real
            # work is delayed by at most one of them.
            warm = const.tile([6, P], bf16)
            nc.vector.memset(warm[:], 0.0)
            ones = const.tile([P, 1], bf16)
            nc.vector.memset(ones[:], 1.0)
            wp = pwarm.tile([P, P], f32, tag="warm")
            for _ in range(16):
                nc.tensor.matmul(wp[:], warm[:], warm[:], tile_position=(0, 0))

            sa = const.tile([P, NT], f32)    # ACT partial row-sums
            acc = pacc.tile([P, NT], f32)    # PE-accumulated Schraudolph sums

            def fcol(c4):  # start column of quad c4's features
                return ca if c4 == 0 else 3 * HALF + (c4 - 1) * QS

            t = 0
            pend_ones = None
            for c4 in range(GSH // QS):       # 4 quads per group
                for g in range(NG):
                    gp = 32 * g
                    tp = (gp, 0)
                    feat = blob[gp:gp + 6, fcol(c4):fcol(c4) + QS]
                    # sample-major matmuls + ACT exp/accum per tile (first:
                    # they feed the bottleneck engine)
                    for j in range(QS // P):
                        lhsT = blob[gp:gp + 6, fcol(c4) + j * P:fcol(c4) + (j + 1) * P]
                        pa = psa.tile([P, ca], f32, tag="pa")
                        nc.tensor.matmul(pa[:], lhsT, blob[gp:gp + 6, 0:ca],
                                         tile_position=tp)
                        nc.scalar.activation(
                            pa[:], pa[:], mybir.ActivationFunctionType.Exp,
                            accum_out=sa[:, t + j:t + j + 1],
                        )
                    # flipped chunk matmuls -> single-chunk psum tiles
                    pbs = []
                    for k in range(NCH):
                        pb = psb.tile([P, QS], f32, tag="pb")
                        cchunk = blob[gp:gp + 6, 2 * HALF + k * P:2 * HALF + (k + 1) * P]
                        nc.tensor.matmul(pb[:], cchunk, feat, tile_position=tp)
                        pbs.append(pb)
                    # DVE Schraudolph: f32 -> bf16 bits (int16)
                    eb = ebuf_pool.tile([P, NCH, QS], i16, tag="eb")
                    for k in range(NCH):
                        nc.vector.tensor_scalar(
                            out=eb[:, k, :], in0=pbs[k][:],
                            scalar1=_SCH_A16, scalar2=_SCH_B16,
                            op0=mybir.AluOpType.mult, op1=mybir.AluOpType.add,
                        )
                    # PE ones-matmuls for the PREVIOUS quad (sems long ready)
                    if pend_ones is not None:
                        peb, pt0 = pend_ones
                        ebv = peb.bitcast(bf16)
                        for j in range(QS // P):
                            for k in range(NCH):
                                nc.tensor.matmul(
                                    acc[:, pt0 + j:pt0 + j + 1],
                                    ebv[:, k, j * P:(j + 1) * P], ones[:],
                                    start=(k == 0), stop=(k == NCH - 1),
                                    tile_position=(0, 0),
                                    skip_group_check=True,
                                )
                    pend_ones = (eb, t)
                    t += QS // P
            peb, pt0 = pend_ones
            ebv = peb.bitcast(bf16)
            for j in range(QS // P):
                for k in range(NCH):
                    nc.tensor.matmul(
                        acc[:, pt0 + j:pt0 + j + 1],
                        ebv[:, k, j * P:(j + 1) * P], ones[:],
                        start=(k == 0), stop=(k == NCH - 1),
                        tile_position=(0, 0),
                        skip_group_check=True,
                    )

            # ship per-sample mixture sums; host does log in f64 (cheaper
            # tail: no Ln table load, no final reduce chain)
            stot = const.tile([P, NT], f32)
            nc.vector.tensor_tensor(
                out=stot[:], in0=sa[:], in1=acc[:], op=mybir.AluOpType.add
            )
            nc.sync.dma_start(out=out_d[:], in_=stot[:])

    nc.compile()
    return nc


def kernel(sample, mu, sigma_log, theta, w):
    import ml_dtypes

    x = sample[:, 0].astype(np.float64)
    y = sample[:, 1].astype(np.float64)
    mux = mu[:, 0].astype(np.float64)
    muy = mu[:, 1].astype(np.float64)
    sl = sigma_log.astype(np.float64)
    th = theta.astype(np.float64)
    wv = w[:, 0].astype(np.float64)

    a = np.exp(-2.0 * sl[:, 0])
    b = np.exp(-2.0 * sl[:, 1])
    c, s = np.cos(th), np.sin(th)
    g11 = a * c * c + b * s * s
    g12 = (a - b) * c * s
    g22 = a * s * s + b * c * c
    wmax = wv.max()
    wlog = (wv - (wmax + np.log(np.exp(wv - wmax).sum()))) - sl.sum(axis=1)

    cm = np.stack([
        wlog - (g11 * mux * mux + 2.0 * g12 * mux * muy + g22 * muy * muy),
        2.0 * (g11 * mux + g12 * muy),
        2.0 * (g12 * mux + g22 * muy),
        -g11,
        -2.0 * g12,
        -g22,
    ]).astype(np.float32)
    ftf = np.stack([np.ones_like(x), x, y, x * x, x * y, y * y]).astype(np.float32)

    cm16 = cm.astype(ml_dtypes.bfloat16)
    ftf16 = ftf.astype(ml_dtypes.bfloat16)

    if "nc" not in _cache:
        _cache["nc"] = _build()
    nc = _cache["nc"]

    in_maps = []
    for i in range(NCORES):
        blob = np.zeros((102, BLOBW), dtype=ml_dtypes.bfloat16)
        base = i * NSH
        for g in range(NG):
            gp = 32 * g
            fg = ftf16[:, base + g * GSH:base + (g + 1) * GSH]
            blob[gp:gp + 6, 0:CA] = cm16[:, 0:CA]              # cmatA
            blob[gp:gp + 6, CA:2 * HALF] = fg[:, 0:QS]         # F quad 0
            blob[gp:gp + 6, 2 * HALF:3 * HALF] = cm16[:, CA:M]  # cmatB
            blob[gp:gp + 6, 3 * HALF:BLOBW] = fg[:, QS:GSH]    # F quads 1-3
        in_maps.append({"blob": blob})
    res = run_bass_kernel_spmd(nc, in_maps, core_ids=list(range(NCORES)))
    _cache["last_result"] = res
    total = np.float64(0.0)
    for r in res.results:
        total += np.log(np.asarray(r["out"], dtype=np.float64)).sum()
    return np.float32(-total)
